# revision 4
# baseline (speedup 1.0000x reference)
"""GRU decoder Trainium2 kernel (data-parallel over batch, 8 cores).

Reference (per step t, PyTorch nn.GRU gate order r,z,n):
    gi = x @ w_ih.T + b_ih ; gh = h @ w_hh.T + b_hh
    r = sig(i_r + h_r); z = sig(i_z + h_z); n = tanh(i_n + r * h_n)
    h' = (1-z)*n + z*h ; y = h' @ w_fc.T + b_fc ; x <- y
Shapes: H=1024, O=768, B=256, T=256.  Each core handles 32 batch rows.

Structure (v7 - rt-inject + half-split pipelined state):
  * x_t = y_{t-1} folds into the hidden-side matmuls, so every recurrent
    matmul contracts over H=1024: regions r, hn (= h_n), z, in (= i_n).
  * State lives ONLY as hsb = h'^T (bf16, PE lhsT layout).
  * rt = sig(r)*hn is ACCUMULATED INTO the gI PSUM by an identity
    matmul (K=128 fp32 lhsT=I), so n = tanh(gI) reads PSUM directly -
    no separate DVE add on the critical tail.
  * gI is split into two per-half PSUM tiles (h0 = feature cols 0:128
    per quadrant = even h-chunks after transpose; h1 = odd chunks).
    tanh / n-transpose / combine all run per-half, and the next step's
    r|hn matmuls are issued even-chunks-first so they only wait on the
    h0 half of the new state.
  * Combine fused via scalar_tensor_tensor:
        q = (z^T - 1) * n^T ;  hsb' = z^T*hsb - q
  * Gate PSUM: pair tile [r|hn] via N=512 matmuls + separate z bank +
    per-half in banks.  Biases seed PSUM via K=1 ones-row matmuls.
    One start=True per bank strip only.
  * Every PSUM tile is padded to 512 f32 so it owns a full bank
    (8 tiles = 8 banks); avoids PE-write/DVE-read same-bank serialization.
  * Step-0 gates come from the host; b_fc is added on the host.
"""

import numpy as np
import ml_dtypes

import concourse.bass as bass
import concourse.bacc as bacc
import concourse.tile as tile
from concourse import mybir
from concourse.bass_utils import run_bass_kernel_spmd

H = 1024
O = 768
B = 256
T = 256
NCORES = 8
BC = B // NCORES  # 32 batch rows per core

KH = H // 128  # 8 contraction chunks
NGATE = 4      # regions r, hn, z, in (issue order)
YW = O // 4    # 192 y cols per quadrant

F32 = mybir.dt.float32
BF16 = mybir.dt.bfloat16
AF = mybir.ActivationFunctionType
ALU = mybir.AluOpType

_COMPILED = None

# bf16 const layout: WG | WF | ONES | BIAS
WG_N = NGATE * KH * 4 * 256   # 32768
WF_N = KH * 4 * YW            # 6144
NB = WG_N + WF_N + 32 + 4096  # 43040
# f32 const layout: G0 (r|hn|z|in) | H0T | IDT | ONESF
NF = NGATE * 256 + 256 + 128 + 256  # 1664

KEVEN = (0, 2, 4, 6)
KODD = (1, 3, 5, 7)


def _hslice(hsb, k):
    """lhsT chunk k (h features 128k..128k+128) from packed h'^T tile."""
    c = 128 * (k % 2) + 32 * (k // 2)
    return hsb[:, c : c + 32]


def _build_nc():
    nc = bacc.Bacc("TRN2", target_bir_lowering=False, debug=False, num_devices=NCORES)

    cb = nc.declare_dram_parameter("CB", [128, NB], BF16, isOutput=False)
    cf = nc.declare_dram_parameter("CF", [128, NF], F32, isOutput=False)
    o = nc.declare_dram_parameter("O", [T, 128, YW], F32, isOutput=True)

    with tile.TileContext(nc) as tc:
        with (
            tc.tile_pool(name="wpool", bufs=1) as wpool,
            tc.tile_pool(name="state", bufs=2) as spool,
            tc.tile_pool(name="act", bufs=2) as apool,
            tc.tile_pool(name="gps", bufs=1, space="PSUM") as gpool,
        ):
            CB = wpool.tile([128, NB], BF16, tag="CB")
            CF = wpool.tile([128, NF], F32, tag="CF")
            nc.sync.dma_start(CB[:], cb[:])
            nc.sync.dma_start(CF[:], cf[:])
            WG = CB[:, 0:WG_N]
            WF = CB[:, WG_N : WG_N + WF_N]
            ONES = CB[0:1, WG_N + WF_N : WG_N + WF_N + 32]
            BIAS = CB[0:1, WG_N + WF_N + 32 : NB]
            G0 = CF[:, 0 : NGATE * 256]
            H0T = CF[:, NGATE * 256 : NGATE * 256 + 256]
            IDT = CF[:, NGATE * 256 + 256 : NGATE * 256 + 384]

            def psum_tile(tag):
                # full-bank (512 f32) tile so no two tiles share a bank
                return gpool.tile([128, 512], F32, tag=tag, name=tag)

            def emit_bias(gA, gZ, gI0, gI1):
                # gA: ONE N=512 MM per quadrant covering r|hn together.
                for j in range(4):
                    nc.tensor.matmul(
                        gA[32 * j : 32 * j + 32, 0:512],
                        ONES[:, 0:32],
                        BIAS[:, 512 * j : 512 * j + 512],
                        start=True, stop=False, tile_position=(0, 32 * j),
                    )
                for j in range(4):
                    nc.tensor.matmul(
                        gZ[32 * j : 32 * j + 32, 0:256],
                        ONES[:, 0:32],
                        BIAS[:, 2048 + 256 * j : 2048 + 256 * j + 256],
                        start=True, stop=False, tile_position=(0, 32 * j),
                    )
                for hh, gt in ((0, gI0), (1, gI1)):
                    for j in range(4):
                        bofs = 3072 + 256 * j + 128 * hh
                        nc.tensor.matmul(
                            gt[32 * j : 32 * j + 32, 0:128],
                            ONES[:, 0:32],
                            BIAS[:, bofs : bofs + 128],
                            start=True, stop=False, tile_position=(0, 32 * j),
                        )

            def emit_A(hsb, gA):
                # r|hn pair as single N=512 matmuls; even chunks first so
                # only the h0 half of the fresh state gates the start.
                for i, k in enumerate(KEVEN + KODD):
                    lhsT = _hslice(hsb, k)
                    for j in range(4):
                        wofs = (k * 4 + j) * 512
                        nc.tensor.matmul(
                            gA[32 * j : 32 * j + 32, 0:512],
                            lhsT,
                            WG[:, wofs : wofs + 512],
                            start=False,
                            stop=(i == KH - 1),
                            tile_position=(0, 32 * j),
                        )

            def emit_Z(hsb, gZ):
                for i, k in enumerate(KEVEN + KODD):
                    lhsT = _hslice(hsb, k)
                    for j in range(4):
                        wofs = 16384 + (k * 4 + j) * 256
                        nc.tensor.matmul(
                            gZ[32 * j : 32 * j + 32, 0:256],
                            lhsT,
                            WG[:, wofs : wofs + 256],
                            start=False,
                            stop=(i == KH - 1),
                            tile_position=(0, 32 * j),
                        )

            def emit_IN_half(hsb, gt, hh):
                # in-region, feature half hh (cols 128*hh of each 256 block)
                for k in range(KH):
                    lhsT = _hslice(hsb, k)
                    for j in range(4):
                        wofs = 16384 + ((KH + k) * 4 + j) * 256 + 128 * hh
                        nc.tensor.matmul(
                            gt[32 * j : 32 * j + 32, 0:128],
                            lhsT,
                            WG[:, wofs : wofs + 128],
                            start=False,
                            stop=False,
                            tile_position=(0, 32 * j),
                        )

            def emit_inject(gt, rt, hh):
                # gI_half += rt[:, 128*hh:128*hh+128] via identity matmul
                nc.tensor.matmul(
                    gt[:, 0:128],
                    IDT,
                    rt[:, 128 * hh : 128 * hh + 128],
                    start=False,
                    stop=True,
                )

            def emit_y(hsb_t, tpY):
                for k in range(KH):
                    lhsT = _hslice(hsb_t, k)
                    for j in range(4):
                        wofs = (k * 4 + j) * YW
                        nc.tensor.matmul(
                            tpY[32 * j : 32 * j + 32, 0:YW],
                            lhsT,
                            WF[:, wofs : wofs + YW],
                            start=(k == 0),
                            stop=(k == KH - 1),
                            tile_position=(0, 32 * j),
                        )

            def combine(tpZ, tpN0, tpN1, hsb_prev):
                """hsb' = z^T*hsb + (1-z^T)*n^T, per half, bf16.
                zcT/pT run early (only need z^T); vT/hsb2 follow each n^T
                transpose.  DVE ops read at most one PSUM operand."""
                hsb2 = spool.tile([128, 256], BF16, tag="hsb")
                zcT = apool.tile([128, 256], F32, tag="zcT")
                nc.vector.tensor_scalar(
                    zcT[:], tpZ[:, 0:256], 1.0, -1.0, ALU.subtract, ALU.mult
                )
                pT0 = apool.tile([128, 128], F32, tag="pT0")
                nc.vector.tensor_tensor(pT0[:], tpZ[:, 0:128], hsb_prev[:, 0:128], ALU.mult)
                pT1 = apool.tile([128, 128], F32, tag="pT1")
                nc.vector.tensor_tensor(pT1[:], tpZ[:, 128:256], hsb_prev[:, 128:256], ALU.mult)
                for hh, tpN in ((0, tpN0), (1, tpN1)):
                    sl = slice(128 * hh, 128 * hh + 128)
                    vT = apool.tile([128, 128], F32, tag=f"vT{hh}")
                    nc.vector.tensor_tensor(vT[:], tpN[:, 0:128], zcT[:, sl], ALU.mult)
                    nc.vector.tensor_tensor(hsb2[:, sl], vT[:], pT0[:] if hh == 0 else pT1[:], ALU.add)
                return hsb2

            # ---- step 0: gates computed host-side (biases included) ----
            rs0 = apool.tile([128, 256], F32, tag="rs")
            nc.scalar.activation(rs0[:], G0[:, 0:256], AF.Sigmoid)
            zs0 = apool.tile([128, 256], F32, tag="zs")
            nc.scalar.activation(zs0[:], G0[:, 512:768], AF.Sigmoid)
            rt0 = apool.tile([128, 256], F32, tag="rt")
            nc.vector.tensor_tensor(rt0[:], rs0[:], G0[:, 256:512], ALU.mult)
            ns0 = apool.tile([128, 256], F32, tag="ns0")
            nc.vector.tensor_tensor(ns0[:], rt0[:], G0[:, 768:1024], ALU.add)
            n0 = apool.tile([128, 256], F32, tag="n0")
            nc.scalar.activation(n0[:], ns0[:], AF.Tanh)
            tpZ0 = psum_tile("tpZ")
            nc.tensor.transpose(tpZ0[:, 0:128], zs0[:, 0:128], IDT)
            nc.tensor.transpose(tpZ0[:, 128:256], zs0[:, 128:256], IDT)
            tpN0_0 = psum_tile("tpN0")
            tpN1_0 = psum_tile("tpN1")
            nc.tensor.transpose(tpN0_0[:, 0:128], n0[:, 0:128], IDT)
            nc.tensor.transpose(tpN1_0[:, 0:128], n0[:, 128:256], IDT)
            hsb = combine(tpZ0, tpN0_0, tpN1_0, H0T)

            for t in range(T):
                last = t == T - 1
                tpY = psum_tile("tpY")
                if not last:
                    # ---- gates for step t+1, read hsb_t ----
                    gA = psum_tile("gA")   # r | hn
                    gZ = psum_tile("gZ")
                    gI0 = psum_tile("gI0")
                    gI1 = psum_tile("gI1")
                    emit_bias(gA, gZ, gI0, gI1)
                    emit_A(hsb, gA)
                    emit_Z(hsb, gZ)

                    rs = apool.tile([128, 256], F32, tag="rs")
                    nc.scalar.activation(rs[:], gA[:, 0:256], AF.Sigmoid)
                    zs = apool.tile([128, 256], F32, tag="zs")
                    nc.scalar.activation(zs[:], gZ[:, 0:256], AF.Sigmoid)
                    rt = apool.tile([128, 256], F32, tag="rt")
                    nc.vector.tensor_tensor(rt[:], rs[:], gA[:, 256:512], ALU.mult)

                    emit_IN_half(hsb, gI0, 0)
                    emit_inject(gI0, rt, 0)
                    emit_IN_half(hsb, gI1, 1)
                    emit_inject(gI1, rt, 1)

                    n_h0 = apool.tile([128, 128], F32, tag="n_h0")
                    nc.scalar.activation(n_h0[:], gI0[:, 0:128], AF.Tanh)
                    n_h1 = apool.tile([128, 128], F32, tag="n_h1")
                    nc.scalar.activation(n_h1[:], gI1[:, 0:128], AF.Tanh)

                    tpZ = psum_tile("tpZ")
                    nc.tensor.transpose(tpZ[:, 0:128], zs[:, 0:128], IDT)
                    nc.tensor.transpose(tpZ[:, 128:256], zs[:, 128:256], IDT)
                    tpN0 = psum_tile("tpN0")
                    nc.tensor.transpose(tpN0[:, 0:128], n_h0[:], IDT)
                    tpN1 = psum_tile("tpN1")
                    nc.tensor.transpose(tpN1[:, 0:128], n_h1[:], IDT)

                    emit_y(hsb, tpY)
                    hsb = combine(tpZ, tpN0, tpN1, hsb[:])
                else:
                    emit_y(hsb, tpY)

                ys = apool.tile([128, YW], F32, tag="ys")
                nc.vector.tensor_copy(ys[:], tpY[:, 0:YW])
                nc.sync.dma_start(o[t], ys[:])

    nc.compile()
    return nc


def _pack_bat(M):
    """[32, 4*W] -> [128, W]: row 32j+b holds M[b, W*j : W*j+W]."""
    w = M.shape[1] // 4
    return np.ascontiguousarray(
        M.reshape(BC, 4, w).transpose(1, 0, 2).reshape(128, w)
    )


def _prep_shared(w_ih, w_hh, b_ih, b_hh, w_fc, b_fc):
    wihT = w_ih.T.astype(np.float64)  # [768, 3072]
    whhT = w_hh.T.astype(np.float64)  # [1024, 3072]
    wfcT = w_fc.T.astype(np.float64)  # [1024, 768]
    fold = wfcT @ wihT                # [1024, 3072]
    Wr = fold[:, 0:H] + whhT[:, 0:H]
    Wz = fold[:, H : 2 * H] + whhT[:, H : 2 * H]
    Win = fold[:, 2 * H : 3 * H]
    Whn = whhT[:, 2 * H : 3 * H]

    bfold = b_fc.astype(np.float64) @ wihT  # [3072]
    br = bfold[0:H] + b_ih[0:H] + b_hh[0:H]
    bz = bfold[H : 2 * H] + b_ih[H : 2 * H] + b_hh[H : 2 * H]
    bin_ = bfold[2 * H :] + b_ih[2 * H :]
    bhn = b_hh[2 * H :].astype(np.float64)

    blocks = []
    # r|hn interleaved per (k,j) for N=512 pair matmuls
    for k in range(KH):
        for j in range(4):
            blocks.append(Wr[128 * k : 128 * k + 128, 256 * j : 256 * j + 256])
            blocks.append(Whn[128 * k : 128 * k + 128, 256 * j : 256 * j + 256])
    # then z, in blocks (N=256)
    for G in (Wz, Win):
        for k in range(KH):
            for j in range(4):
                blocks.append(G[128 * k : 128 * k + 128, 256 * j : 256 * j + 256])
    WGp = np.concatenate(blocks, axis=1).astype(ml_dtypes.bfloat16)  # [128, 32768]

    yblocks = []
    for k in range(KH):
        for j in range(4):
            yblocks.append(wfcT[128 * k : 128 * k + 128, YW * j : YW * j + YW])
    WFp = np.concatenate(yblocks, axis=1).astype(ml_dtypes.bfloat16)  # [128, 6144]

    ones_col = np.zeros((128, 32), ml_dtypes.bfloat16)
    ones_col[0, :] = 1
    # bias layout: j-paired [br_j | bhn_j] (4x512) then bz (1024), bin (1024)
    bias_row = np.empty(4096, np.float64)
    for j in range(4):
        bias_row[512 * j : 512 * j + 256] = br[256 * j : 256 * j + 256]
        bias_row[512 * j + 256 : 512 * j + 512] = bhn[256 * j : 256 * j + 256]
    bias_row[2048:3072] = bz
    bias_row[3072:4096] = bin_
    bias_col = np.zeros((128, 4096), ml_dtypes.bfloat16)
    bias_col[0, :] = bias_row.astype(ml_dtypes.bfloat16)

    CBp = np.concatenate([WGp, WFp, ones_col, bias_col], axis=1)  # [128, NB]
    assert CBp.shape[1] == NB
    IDT = np.eye(128, dtype=np.float32)
    return CBp, IDT


def _build_in_maps(inputs):
    src = np.asarray(inputs["src"], np.float32)
    hidden = np.asarray(inputs["hidden"], np.float32)
    w_ih = np.asarray(inputs["w_ih"], np.float32)
    w_hh = np.asarray(inputs["w_hh"], np.float32)
    b_ih = np.asarray(inputs["b_ih"], np.float32)
    b_hh = np.asarray(inputs["b_hh"], np.float32)
    w_fc = np.asarray(inputs["w_fc"], np.float32)
    b_fc = np.asarray(inputs["b_fc"], np.float32)

    CBp, IDT = _prep_shared(w_ih, w_hh, b_ih, b_hh, w_fc, b_fc)

    # step-0 gates on host (f64): from x0=src[0], h0=hidden[0]
    x0 = src[0].astype(np.float64)
    h0 = hidden[0].astype(np.float64)
    gi0 = x0 @ w_ih.T.astype(np.float64) + b_ih.astype(np.float64)
    gh0 = h0 @ w_hh.T.astype(np.float64) + b_hh.astype(np.float64)
    g0r = gi0[:, 0:H] + gh0[:, 0:H]
    g0z = gi0[:, H : 2 * H] + gh0[:, H : 2 * H]
    g0in = gi0[:, 2 * H :]
    g0hn = gh0[:, 2 * H :]

    in_maps = []
    for c in range(NCORES):
        sl = slice(BC * c, BC * (c + 1))
        G0 = np.concatenate(
            [
                _pack_bat(g0r[sl]),
                _pack_bat(g0hn[sl]),
                _pack_bat(g0z[sl]),
                _pack_bat(g0in[sl]),
            ],
            axis=1,
        )  # [128, 1024] in region order r|hn|z|in
        HP0 = _pack_bat(h0[sl])  # [128, 256]
        H0T = np.concatenate(
            [HP0[:, 0:128].T, HP0[:, 128:256].T], axis=1
        )  # transposed-state layout
        CFp = np.concatenate([G0, H0T, IDT, np.ones((128, 256), np.float32)], axis=1).astype(np.float32)
        assert CFp.shape[1] == NF
        in_maps.append(dict(CB=CBp, CF=CFp))
    return in_maps


def kernel(src, tgt, hidden, w_ih, w_hh, b_ih, b_hh, w_fc, b_fc, **_kw):
    global _COMPILED
    b_fc = np.asarray(b_fc, np.float32)

    if _COMPILED is None:
        _COMPILED = _build_nc()
    nc = _COMPILED

    in_maps = _build_in_maps(
        dict(src=src, hidden=hidden, w_ih=w_ih, w_hh=w_hh, b_ih=b_ih,
             b_hh=b_hh, w_fc=w_fc, b_fc=b_fc)
    )

    res = run_bass_kernel_spmd(nc, in_maps, list(range(NCORES)))

    out = np.empty((T, B, O), np.float32)
    for c in range(NCORES):
        sl = slice(BC * c, BC * (c + 1))
        oc = np.asarray(res.results[c]["O"])  # [T, 128, 192]
        out[:, sl, :] = (
            oc.reshape(T, 4, BC, YW).transpose(0, 2, 1, 3).reshape(T, BC, O)
        )
    out += b_fc[None, None, :]
    return out


# revision 10
# speedup vs baseline: 1.0778x; 1.0778x over previous
"""GRU decoder Trainium2 kernel (data-parallel over batch, 8 cores).

Reference (per step t, PyTorch nn.GRU gate order r,z,n):
    gi = x @ w_ih.T + b_ih ; gh = h @ w_hh.T + b_hh
    r = sig(i_r + h_r); z = sig(i_z + h_z); n = tanh(i_n + r * h_n)
    h' = (1-z)*n + z*h ; y = h' @ w_fc.T + b_fc ; x <- y
Shapes: H=1024, O=768, B=256, T=256.  Each core handles 32 batch rows.

Structure (v8 - transposed state + rt PSUM-inject):
  * x_t = y_{t-1} folds into the hidden-side matmuls, so every recurrent
    matmul contracts over H=1024: regions r, hn (= h_n), z, in (= i_n).
  * The state lives ONLY as hsb = h'^T (bf16, PE lhsT layout).  The
    chain computes zs/n in normal layout, transposes zs and n (PE,
    cheap, off the critical tail), then finishes in transposed space:
        hsb' = n^T (1 - z^T) + z^T hsb
    so NOTHING follows the last vector op before the next gate matmuls.
  * Gate PSUM: pair tile [r|hn] computed with single N=512 matmuls
    (fewer LDWEIGHTS/issues) + separate z / in banks (separate banks =
    per-region dependency granularity, so zs runs before the in-region
    finishes), all double-buffered.
  * Biases seed PSUM via K=1 ones-row matmuls issued in the PE-idle
    chain window.  One start=True per bank strip only - a second start
    in the same strip clears has_written and loses the earlier bias.
  * y_t matmuls + zs/n transposes share PSUM banks with the chain
    scratch; y and bias MMs hide in the chain window.
  * Step-0 gates come from the host; b_fc is added on the host.
"""

import numpy as np
import ml_dtypes

import concourse.bass as bass
import concourse.bacc as bacc
import concourse.tile as tile
from concourse import mybir
from concourse.bass_utils import run_bass_kernel_spmd

H = 1024
O = 768
B = 256
T = 256
NCORES = 8
BC = B // NCORES  # 32 batch rows per core

KH = H // 128  # 8 contraction chunks
NGATE = 4      # regions r, hn, z, in (issue order)
YW = O // 4    # 192 y cols per quadrant

F32 = mybir.dt.float32
BF16 = mybir.dt.bfloat16
AF = mybir.ActivationFunctionType
ALU = mybir.AluOpType

_COMPILED = None

# bf16 const layout: WG | WF | ONES | BIAS
WG_N = NGATE * KH * 4 * 256   # 32768
WF_N = KH * 4 * YW            # 6144
NB = WG_N + WF_N + 32 + 4096  # 43040
# f32 const layout: G0 (r|hn|z|in) | H0T | IDT | ONESF
NF = NGATE * 256 + 256 + 128 + 256  # 1664


def _hslice(hsb, k):
    """lhsT chunk k (h features 128k..128k+128) from packed h'^T tile."""
    c = 128 * (k % 2) + 32 * (k // 2)
    return hsb[:, c : c + 32]


def _build_nc():
    nc = bacc.Bacc("TRN2", target_bir_lowering=False, debug=False, num_devices=NCORES)

    cb = nc.declare_dram_parameter("CB", [128, NB], BF16, isOutput=False)
    cf = nc.declare_dram_parameter("CF", [128, NF], F32, isOutput=False)
    o = nc.declare_dram_parameter("O", [T, 128, YW], F32, isOutput=True)

    with tile.TileContext(nc) as tc:
        with (
            tc.tile_pool(name="wpool", bufs=1) as wpool,
            tc.tile_pool(name="state", bufs=2) as spool,
            tc.tile_pool(name="act", bufs=2) as apool,
            tc.tile_pool(name="gps", bufs=2, space="PSUM") as gpool,
            tc.tile_pool(name="tps", bufs=1, space="PSUM") as tpool,
        ):
            CB = wpool.tile([128, NB], BF16, tag="CB")
            CF = wpool.tile([128, NF], F32, tag="CF")
            nc.sync.dma_start(CB[:], cb[:])
            nc.sync.dma_start(CF[:], cf[:])
            WG = CB[:, 0:WG_N]
            WF = CB[:, WG_N : WG_N + WF_N]
            ONES = CB[0:1, WG_N + WF_N : WG_N + WF_N + 32]
            BIAS = CB[0:1, WG_N + WF_N + 32 : NB]
            G0 = CF[:, 0 : NGATE * 256]
            H0T = CF[:, NGATE * 256 : NGATE * 256 + 256]
            IDT = CF[:, NGATE * 256 + 256 : NGATE * 256 + 384]
            ONESF = CF[:, NGATE * 256 + 384 : NF]  # all-ones f32 [128,256]

            def chain_partA(r_src, hn_src, z_src, in_src):
                """Step-0 variant (all-SBUF sources): scalar: rs, zs, tanh;
                vector: rt, ns; PE: zs^T."""
                rs = apool.tile([128, 256], F32, tag="rs")
                nc.scalar.activation(rs[:], r_src, AF.Sigmoid)
                zs = apool.tile([128, 256], F32, tag="zs")
                nc.scalar.activation(zs[:], z_src, AF.Sigmoid)
                rt = apool.tile([128, 256], F32, tag="rt")
                nc.vector.tensor_tensor(rt[:], rs[:], hn_src, ALU.mult)
                ns = apool.tile([128, 256], F32, tag="ns")
                nc.vector.tensor_tensor(ns[:], rt[:], in_src, ALU.add)
                tpZ = tpool.tile([128, 256], F32, tag="tpZ")
                nc.tensor.transpose(tpZ[:, 0:128], zs[:, 0:128], IDT)
                nc.tensor.transpose(tpZ[:, 128:256], zs[:, 128:256], IDT)
                n = apool.tile([128, 256], F32, tag="n")
                nc.scalar.activation(n[:], ns[:], AF.Tanh)
                tpN = tpool.tile([128, 448], F32, tag="tpN")
                return n, tpZ, tpN

            def chain_partB(n, tpZ, tpN, hsb_prev):
                """PE: n^T; vector: zc^T (= 1 - z^T), p^T, v^T, hsb' (bf16)."""
                nc.tensor.transpose(tpN[:, 0:128], n[:, 0:128], IDT)
                nc.tensor.transpose(tpN[:, 128:256], n[:, 128:256], IDT)
                zcT = apool.tile([128, 256], F32, tag="zcT")
                nc.vector.tensor_tensor(zcT[:], ONESF, tpZ[:], ALU.subtract)
                pT = apool.tile([128, 256], F32, tag="pT")
                nc.vector.tensor_tensor(pT[:], tpZ[:], hsb_prev, ALU.mult)
                vT = apool.tile([128, 256], F32, tag="vT")
                nc.vector.tensor_tensor(vT[:], tpN[:, 0:256], zcT[:], ALU.mult)
                hsb2 = spool.tile([128, 256], BF16, tag="hsb")
                nc.vector.tensor_tensor(hsb2[:], vT[:], pT[:], ALU.add)
                return hsb2

            def emit_y(hsb_t, tpN):
                for k in range(KH):
                    lhsT = _hslice(hsb_t, k)
                    for j in range(4):
                        wofs = (k * 4 + j) * YW
                        nc.tensor.matmul(
                            tpN[32 * j : 32 * j + 32, 256:448],
                            lhsT,
                            WF[:, wofs : wofs + YW],
                            start=(k == 0),
                            stop=(k == KH - 1),
                            tile_position=(0, 32 * j),
                        )

            # step 0: gates computed host-side (biases already included)
            n0, tpZ0, tpN0 = chain_partA(
                G0[:, 0:256], G0[:, 256:512], G0[:, 512:768], G0[:, 768:1024]
            )
            hsb = chain_partB(n0, tpZ0, tpN0, H0T)

            for t in range(T):
                last = t == T - 1
                if not last:
                    # gates for step t+1, read hsb_t
                    gA = gpool.tile([128, 512], F32, tag="gA")  # r | hn
                    gZ = gpool.tile([128, 256], F32, tag="gZ")
                    gI = gpool.tile([128, 256], F32, tag="gI")
                    # bias seeds (start=True).  gA: ONE N=512 MM per
                    # quadrant covering r|hn together (a second start in
                    # the same bank strip would clear the earlier bias).
                    for j in range(4):
                        nc.tensor.matmul(
                            gA[32 * j : 32 * j + 32, :],
                            ONES[:, 0:32],
                            BIAS[:, 512 * j : 512 * j + 512],
                            start=True, stop=False, tile_position=(0, 32 * j),
                        )
                    for gi, gt in ((2, gZ), (3, gI)):
                        for j in range(4):
                            bofs = 1024 * gi + 256 * j
                            nc.tensor.matmul(
                                gt[32 * j : 32 * j + 32, :],
                                ONES[:, 0:32],
                                BIAS[:, bofs : bofs + 256],
                                start=True, stop=False, tile_position=(0, 32 * j),
                            )
                    # r|hn pair as single N=512 matmuls (fewer LDW/issues)
                    for k in range(KH):
                        lhsT = _hslice(hsb, k)
                        for j in range(4):
                            wofs = (k * 4 + j) * 512
                            nc.tensor.matmul(
                                gA[32 * j : 32 * j + 32, :],
                                lhsT,
                                WG[:, wofs : wofs + 512],
                                start=False,
                                stop=(k == KH - 1),
                                tile_position=(0, 32 * j),
                            )
                    # z region (N=256, own bank -> zs can run before the
                    # in-region finishes)
                    for k in range(KH):
                        lhsT = _hslice(hsb, k)
                        for j in range(4):
                            wofs = 16384 + (k * 4 + j) * 256
                            nc.tensor.matmul(
                                gZ[32 * j : 32 * j + 32, :],
                                lhsT,
                                WG[:, wofs : wofs + 256],
                                start=False,
                                stop=(k == KH - 1),
                                tile_position=(0, 32 * j),
                            )
                    # rs / zs / rt emitted before the in-region so rt is
                    # ready for the PSUM-inject matmuls below.
                    rs = apool.tile([128, 256], F32, tag="rs")
                    nc.scalar.activation(rs[:], gA[:, 0:256], AF.Sigmoid)
                    zs = apool.tile([128, 256], F32, tag="zs")
                    nc.scalar.activation(zs[:], gZ[:], AF.Sigmoid)
                    rt = apool.tile([128, 256], F32, tag="rt")
                    nc.vector.tensor_tensor(rt[:], rs[:], gA[:, 256:512], ALU.mult)
                    # in region (stop on the inject matmuls)
                    for k in range(KH):
                        lhsT = _hslice(hsb, k)
                        for j in range(4):
                            wofs = 16384 + ((KH + k) * 4 + j) * 256
                            nc.tensor.matmul(
                                gI[32 * j : 32 * j + 32, :],
                                lhsT,
                                WG[:, wofs : wofs + 256],
                                start=False,
                                stop=False,
                                tile_position=(0, 32 * j),
                            )
                    # inject rt into gI: gI += I32^T @ rt (per quadrant),
                    # so n = tanh(gI) reads PSUM directly - no DVE add on
                    # the critical tail.
                    for j in range(4):
                        # diagonal I32 block of IDT: operands and PE row
                        # tile must share the same start partition 32j, so
                        # each quadrant uses the diagonal (32j, 32j) PE tile
                        nc.tensor.matmul(
                            gI[32 * j : 32 * j + 32, :],
                            IDT[32 * j : 32 * j + 32, 32 * j : 32 * j + 32],
                            rt[32 * j : 32 * j + 32, :],
                            start=False,
                            stop=True,
                            tile_position=(32 * j, 32 * j),
                        )
                    tpZA = tpool.tile([128, 256], F32, tag="tpZ")
                    nc.tensor.transpose(tpZA[:, 0:128], zs[:, 0:128], IDT)
                    nc.tensor.transpose(tpZA[:, 128:256], zs[:, 128:256], IDT)
                    nA = apool.tile([128, 256], F32, tag="n")
                    nc.scalar.activation(nA[:], gI[:], AF.Tanh)
                    tpNA = tpool.tile([128, 448], F32, tag="tpN")
                    emit_y(hsb, tpNA)
                    hsb = chain_partB(nA, tpZA, tpNA, hsb[:])
                    ysrc = tpNA
                else:
                    tpN_last = tpool.tile([128, 448], F32, tag="tpN")
                    emit_y(hsb, tpN_last)
                    ysrc = tpN_last

                ys = apool.tile([128, YW], F32, tag="ys")
                nc.vector.tensor_copy(ys[:], ysrc[:, 256:448])
                nc.sync.dma_start(o[t], ys[:])

    nc.compile()
    return nc


def _pack_bat(M):
    """[32, 4*W] -> [128, W]: row 32j+b holds M[b, W*j : W*j+W]."""
    w = M.shape[1] // 4
    return np.ascontiguousarray(
        M.reshape(BC, 4, w).transpose(1, 0, 2).reshape(128, w)
    )


def _prep_shared(w_ih, w_hh, b_ih, b_hh, w_fc, b_fc):
    wihT = w_ih.T.astype(np.float64)  # [768, 3072]
    whhT = w_hh.T.astype(np.float64)  # [1024, 3072]
    wfcT = w_fc.T.astype(np.float64)  # [1024, 768]
    fold = wfcT @ wihT                # [1024, 3072]
    Wr = fold[:, 0:H] + whhT[:, 0:H]
    Wz = fold[:, H : 2 * H] + whhT[:, H : 2 * H]
    Win = fold[:, 2 * H : 3 * H]
    Whn = whhT[:, 2 * H : 3 * H]

    bfold = b_fc.astype(np.float64) @ wihT  # [3072]
    br = bfold[0:H] + b_ih[0:H] + b_hh[0:H]
    bz = bfold[H : 2 * H] + b_ih[H : 2 * H] + b_hh[H : 2 * H]
    bin_ = bfold[2 * H :] + b_ih[2 * H :]
    bhn = b_hh[2 * H :].astype(np.float64)

    blocks = []
    # r|hn interleaved per (k,j) for N=512 pair matmuls
    for k in range(KH):
        for j in range(4):
            blocks.append(Wr[128 * k : 128 * k + 128, 256 * j : 256 * j + 256])
            blocks.append(Whn[128 * k : 128 * k + 128, 256 * j : 256 * j + 256])
    # then z, in blocks (N=256)
    for G in (Wz, Win):
        for k in range(KH):
            for j in range(4):
                blocks.append(G[128 * k : 128 * k + 128, 256 * j : 256 * j + 256])
    WGp = np.concatenate(blocks, axis=1).astype(ml_dtypes.bfloat16)  # [128, 32768]

    yblocks = []
    for k in range(KH):
        for j in range(4):
            yblocks.append(wfcT[128 * k : 128 * k + 128, YW * j : YW * j + YW])
    WFp = np.concatenate(yblocks, axis=1).astype(ml_dtypes.bfloat16)  # [128, 6144]

    ones_col = np.zeros((128, 32), ml_dtypes.bfloat16)
    ones_col[0, :] = 1
    # bias layout: j-paired [br_j | bhn_j] (4x512) then bz (1024), bin (1024)
    bias_row = np.empty(4096, np.float64)
    for j in range(4):
        bias_row[512 * j : 512 * j + 256] = br[256 * j : 256 * j + 256]
        bias_row[512 * j + 256 : 512 * j + 512] = bhn[256 * j : 256 * j + 256]
    bias_row[2048:3072] = bz
    bias_row[3072:4096] = bin_
    bias_col = np.zeros((128, 4096), ml_dtypes.bfloat16)
    bias_col[0, :] = bias_row.astype(ml_dtypes.bfloat16)

    CBp = np.concatenate([WGp, WFp, ones_col, bias_col], axis=1)  # [128, NB]
    assert CBp.shape[1] == NB
    IDT = np.eye(128, dtype=np.float32)
    return CBp, IDT


def _build_in_maps(inputs):
    src = np.asarray(inputs["src"], np.float32)
    hidden = np.asarray(inputs["hidden"], np.float32)
    w_ih = np.asarray(inputs["w_ih"], np.float32)
    w_hh = np.asarray(inputs["w_hh"], np.float32)
    b_ih = np.asarray(inputs["b_ih"], np.float32)
    b_hh = np.asarray(inputs["b_hh"], np.float32)
    w_fc = np.asarray(inputs["w_fc"], np.float32)
    b_fc = np.asarray(inputs["b_fc"], np.float32)

    CBp, IDT = _prep_shared(w_ih, w_hh, b_ih, b_hh, w_fc, b_fc)

    # step-0 gates on host (f64): from x0=src[0], h0=hidden[0]
    x0 = src[0].astype(np.float64)
    h0 = hidden[0].astype(np.float64)
    gi0 = x0 @ w_ih.T.astype(np.float64) + b_ih.astype(np.float64)
    gh0 = h0 @ w_hh.T.astype(np.float64) + b_hh.astype(np.float64)
    g0r = gi0[:, 0:H] + gh0[:, 0:H]
    g0z = gi0[:, H : 2 * H] + gh0[:, H : 2 * H]
    g0in = gi0[:, 2 * H :]
    g0hn = gh0[:, 2 * H :]

    in_maps = []
    for c in range(NCORES):
        sl = slice(BC * c, BC * (c + 1))
        G0 = np.concatenate(
            [
                _pack_bat(g0r[sl]),
                _pack_bat(g0hn[sl]),
                _pack_bat(g0z[sl]),
                _pack_bat(g0in[sl]),
            ],
            axis=1,
        )  # [128, 1024] in region order r|hn|z|in
        HP0 = _pack_bat(h0[sl])  # [128, 256]
        H0T = np.concatenate(
            [HP0[:, 0:128].T, HP0[:, 128:256].T], axis=1
        )  # transposed-state layout
        CFp = np.concatenate([G0, H0T, IDT, np.ones((128, 256), np.float32)], axis=1).astype(np.float32)
        assert CFp.shape[1] == NF
        in_maps.append(dict(CB=CBp, CF=CFp))
    return in_maps


def kernel(src, tgt, hidden, w_ih, w_hh, b_ih, b_hh, w_fc, b_fc, **_kw):
    global _COMPILED
    b_fc = np.asarray(b_fc, np.float32)

    if _COMPILED is None:
        _COMPILED = _build_nc()
    nc = _COMPILED

    in_maps = _build_in_maps(
        dict(src=src, hidden=hidden, w_ih=w_ih, w_hh=w_hh, b_ih=b_ih,
             b_hh=b_hh, w_fc=w_fc, b_fc=b_fc)
    )

    res = run_bass_kernel_spmd(nc, in_maps, list(range(NCORES)))

    out = np.empty((T, B, O), np.float32)
    for c in range(NCORES):
        sl = slice(BC * c, BC * (c + 1))
        oc = np.asarray(res.results[c]["O"])  # [T, 128, 192]
        out[:, sl, :] = (
            oc.reshape(T, 4, BC, YW).transpose(0, 2, 1, 3).reshape(T, BC, O)
        )
    out += b_fc[None, None, :]
    return out



# revision 12
# speedup vs baseline: 1.1549x; 1.0715x over previous
"""GRU decoder Trainium2 kernel (data-parallel over batch, 8 cores).

Reference (per step t, PyTorch nn.GRU gate order r,z,n):
    gi = x @ w_ih.T + b_ih ; gh = h @ w_hh.T + b_hh
    r = sig(i_r + h_r); z = sig(i_z + h_z); n = tanh(i_n + r * h_n)
    h' = (1-z)*n + z*h ; y = h' @ w_fc.T + b_fc ; x <- y
Shapes: H=1024, O=768, B=256, T=256.  Each core handles 32 batch rows.

Structure (v9 - rt PSUM-inject + halved tail + evens-first A):
  * x_t = y_{t-1} folds into the hidden-side matmuls, so every recurrent
    matmul contracts over H=1024: regions r, hn (= h_n), z, in (= i_n).
  * The state lives ONLY as hsb = h'^T (bf16, PE lhsT layout).
  * rt = sig(r)*hn (bf16) is ACCUMULATED INTO the gI PSUM by per-quadrant
    diagonal-identity matmuls, so n = tanh(gI) reads PSUM directly - no
    DVE add / extra sem hop on the critical tail.  bf16 rt keeps the
    inject to one cheap round (fp32 would lower to 2 LOW_HIGH passes).
  * The n-side tail is HALVED: tanh / n^T-transpose / vT / hsb' run per
    128-col half into per-bank PSUM tiles, and the next step's r|hn
    matmuls are issued even-chunks-first (even chunks only read the h0
    half of the fresh state), so they start one half earlier.
  * y_t writes its own PSUM bank (no DVE-read-vs-PE-write bank stall on
    the combine) and is split around the n^T transposes to fill PE gaps.
  * Biases seed PSUM via K=1 ones-row matmuls.  One start=True per bank.
  * Step-0 gates come from the host; b_fc is added on the host.
"""

import numpy as np
import ml_dtypes

import concourse.bass as bass
import concourse.bacc as bacc
import concourse.tile as tile
from concourse import mybir
from concourse.bass_utils import run_bass_kernel_spmd

H = 1024
O = 768
B = 256
T = 256
NCORES = 8
BC = B // NCORES  # 32 batch rows per core

KH = H // 128  # 8 contraction chunks
NGATE = 4      # regions r, hn, z, in (issue order)
YW = O // 4    # 192 y cols per quadrant

F32 = mybir.dt.float32
BF16 = mybir.dt.bfloat16
AF = mybir.ActivationFunctionType
ALU = mybir.AluOpType

_COMPILED = None

# bf16 const layout: WG | WF | ONES | BIAS | IB32
WG_N = NGATE * KH * 4 * 256   # 32768
WF_N = KH * 4 * YW            # 6144
NB = WG_N + WF_N + 32 + 4096 + 32  # 43072
# f32 const layout: G0 (r|hn|z|in) | H0T | IDT | ONESF
NF = NGATE * 256 + 256 + 128 + 256  # 1664

KEVEN = (0, 2, 4, 6)
KODD = (1, 3, 5, 7)


def _hslice(hsb, k):
    """lhsT chunk k (h features 128k..128k+128) from packed h'^T tile."""
    c = 128 * (k % 2) + 32 * (k // 2)
    return hsb[:, c : c + 32]


def _build_nc():
    nc = bacc.Bacc("TRN2", target_bir_lowering=False, debug=False, num_devices=NCORES)

    cb = nc.declare_dram_parameter("CB", [128, NB], BF16, isOutput=False)
    cf = nc.declare_dram_parameter("CF", [128, NF], F32, isOutput=False)
    o = nc.declare_dram_parameter("O", [T, 128, YW], F32, isOutput=True)

    with tile.TileContext(nc) as tc:
        with (
            tc.tile_pool(name="wpool", bufs=1) as wpool,
            tc.tile_pool(name="state", bufs=2) as spool,
            tc.tile_pool(name="act", bufs=2) as apool,
            tc.tile_pool(name="gps", bufs=1, space="PSUM") as gpool,
        ):
            CB = wpool.tile([128, NB], BF16, tag="CB")
            CF = wpool.tile([128, NF], F32, tag="CF")
            nc.sync.dma_start(CB[:], cb[:])
            nc.sync.dma_start(CF[:], cf[:])
            WG = CB[:, 0:WG_N]
            WF = CB[:, WG_N : WG_N + WF_N]
            ONES = CB[0:1, WG_N + WF_N : WG_N + WF_N + 32]
            BIAS = CB[0:1, WG_N + WF_N + 32 : WG_N + WF_N + 32 + 4096]
            IB32 = CB[:, WG_N + WF_N + 32 + 4096 : NB]  # 4x vertically tiled I32
            G0 = CF[:, 0 : NGATE * 256]
            H0T = CF[:, NGATE * 256 : NGATE * 256 + 256]
            IDT = CF[:, NGATE * 256 + 256 : NGATE * 256 + 384]
            ONESF = CF[:, NGATE * 256 + 384 : NF]  # all-ones f32 [128,256]

            # PSUM: 8 banks exactly: gA x2 | gZ | gI | tpZ | tpN0 | tpN1 | tpY
            def mk_gA():
                gA = gpool.tile([128, 512], F32, tag="gA", name="gA", bufs=2)
                return gA

            def mk(tag, n):
                # bank-padded (512 f32) so no two PSUM tiles share a bank;
                # hand back a view of the first n cols
                full = gpool.tile([128, 512], F32, tag=tag, name=tag, bufs=1)
                return full[:, 0:n]

            def emit_bias(gA, gZ, gI):
                for j in range(4):
                    nc.tensor.matmul(
                        gA[32 * j : 32 * j + 32, :],
                        ONES[:, 0:32],
                        BIAS[:, 512 * j : 512 * j + 512],
                        start=True, stop=False, tile_position=(0, 32 * j),
                    )
                for gi, gt in ((2, gZ), (3, gI)):
                    for j in range(4):
                        bofs = 1024 * gi + 256 * j
                        nc.tensor.matmul(
                            gt[32 * j : 32 * j + 32, :],
                            ONES[:, 0:32],
                            BIAS[:, bofs : bofs + 256],
                            start=True, stop=False, tile_position=(0, 32 * j),
                        )

            def emit_A(hsb, gA):
                # r|hn pair as single N=512 matmuls; even chunks first so
                # the round 0 only waits on the h0 half of the new state.
                for i, k in enumerate(KEVEN + KODD):
                    lhsT = _hslice(hsb, k)
                    for j in range(4):
                        wofs = (k * 4 + j) * 512
                        nc.tensor.matmul(
                            gA[32 * j : 32 * j + 32, :],
                            lhsT,
                            WG[:, wofs : wofs + 512],
                            start=False,
                            stop=(i == KH - 1),
                            tile_position=(0, 32 * j),
                        )

            def emit_ZI(hsb, gt, gi, stop_last):
                for k in range(KH):
                    lhsT = _hslice(hsb, k)
                    for j in range(4):
                        wofs = 16384 + ((gi * KH + k) * 4 + j) * 256
                        nc.tensor.matmul(
                            gt[32 * j : 32 * j + 32, :],
                            lhsT,
                            WG[:, wofs : wofs + 256],
                            start=False,
                            stop=(stop_last and k == KH - 1),
                            tile_position=(0, 32 * j),
                        )

            def emit_inject(gI, rtb):
                # gI += I32^T @ rt per quadrant (bf16, diagonal PE tiles)
                for j in range(4):
                    nc.tensor.matmul(
                        gI[32 * j : 32 * j + 32, :],
                        IB32[32 * j : 32 * j + 32, 0:32],
                        rtb[32 * j : 32 * j + 32, :],
                        start=False,
                        stop=True,
                        tile_position=(32 * j, 32 * j),
                    )

            def emit_y(hsb_t, tpY, kset):
                for k in kset:
                    lhsT = _hslice(hsb_t, k)
                    for j in range(4):
                        wofs = (k * 4 + j) * YW
                        nc.tensor.matmul(
                            tpY[32 * j : 32 * j + 32, 0:YW],
                            lhsT,
                            WF[:, wofs : wofs + YW],
                            start=(k == 0),
                            stop=(k == KH - 1),
                            tile_position=(0, 32 * j),
                        )

            def chain_zside(zs, tpZ, hsb_prev):
                """zs^T (PE) was emitted by caller; zc^T and p^T on DVE
                (early, off the critical tail)."""
                zcT = apool.tile([128, 256], F32, tag="zcT")
                nc.vector.tensor_tensor(zcT[:], ONESF, tpZ[:], ALU.subtract)
                pT = apool.tile([128, 256], F32, tag="pT")
                nc.vector.tensor_tensor(pT[:], tpZ[:], hsb_prev, ALU.mult)
                return zcT, pT

            def chain_nhalf(tpN, zcT, pT, hsb2, hh):
                """vT = n^T * zc^T ; hsb'[half] = vT + pT[half]  (DVE)."""
                sl = slice(128 * hh, 128 * hh + 128)
                vT = apool.tile([128, 128], F32, tag=f"vT{hh}")
                nc.vector.tensor_tensor(vT[:], tpN[:, 0:128], zcT[:, sl], ALU.mult)
                nc.vector.tensor_tensor(hsb2[:, sl], vT[:], pT[:, sl], ALU.add)

            # ---- step 0: gates computed host-side (biases included) ----
            rs0 = apool.tile([128, 256], F32, tag="rs")
            nc.scalar.activation(rs0[:], G0[:, 0:256], AF.Sigmoid)
            zs0 = apool.tile([128, 256], F32, tag="zs")
            nc.scalar.activation(zs0[:], G0[:, 512:768], AF.Sigmoid)
            rt0 = apool.tile([128, 256], F32, tag="rt0")
            nc.vector.tensor_tensor(rt0[:], rs0[:], G0[:, 256:512], ALU.mult)
            ns0 = apool.tile([128, 256], F32, tag="ns0")
            nc.vector.tensor_tensor(ns0[:], rt0[:], G0[:, 768:1024], ALU.add)
            n0 = apool.tile([128, 256], F32, tag="n0")
            nc.scalar.activation(n0[:], ns0[:], AF.Tanh)
            tpZ = mk("tpZ", 256)
            nc.tensor.transpose(tpZ[:, 0:128], zs0[:, 0:128], IDT)
            nc.tensor.transpose(tpZ[:, 128:256], zs0[:, 128:256], IDT)
            tpN0 = mk("tpN0", 128)
            tpN1 = mk("tpN1", 128)
            nc.tensor.transpose(tpN0[:, 0:128], n0[:, 0:128], IDT)
            nc.tensor.transpose(tpN1[:, 0:128], n0[:, 128:256], IDT)
            zcT, pT = chain_zside(zs0, tpZ, H0T)
            hsb = spool.tile([128, 256], BF16, tag="hsb")
            chain_nhalf(tpN0, zcT, pT, hsb, 0)
            chain_nhalf(tpN1, zcT, pT, hsb, 1)

            for t in range(T):
                last = t == T - 1
                tpY = mk("tpY", YW)
                if not last:
                    gA = mk_gA()         # r | hn
                    gZ = mk("gZ", 256)
                    gI = mk("gI", 256)
                    emit_bias(gA, gZ, gI)
                    emit_A(hsb, gA)
                    emit_ZI(hsb, gZ, 0, stop_last=True)

                    rs = apool.tile([128, 256], F32, tag="rs")
                    nc.scalar.activation(rs[:], gA[:, 0:256], AF.Sigmoid)
                    zs = apool.tile([128, 256], F32, tag="zs")
                    nc.scalar.activation(zs[:], gZ[:], AF.Sigmoid)
                    rtb = apool.tile([128, 256], BF16, tag="rtb")
                    nc.vector.tensor_tensor(rtb[:], rs[:], gA[:, 256:512], ALU.mult)

                    emit_ZI(hsb, gI, 1, stop_last=False)
                    emit_inject(gI, rtb)

                    tpZ = mk("tpZ", 256)
                    nc.tensor.transpose(tpZ[:, 0:128], zs[:, 0:128], IDT)
                    nc.tensor.transpose(tpZ[:, 128:256], zs[:, 128:256], IDT)

                    n_h0 = apool.tile([128, 128], F32, tag="n_h0")
                    nc.scalar.activation(n_h0[:], gI[:, 0:128], AF.Tanh)
                    n_h1 = apool.tile([128, 128], F32, tag="n_h1")
                    nc.scalar.activation(n_h1[:], gI[:, 128:256], AF.Tanh)

                    zcT, pT = chain_zside(zs, tpZ, hsb[:])

                    # PE: fill with half of y while tanh runs, then the
                    # n^T transposes, then the rest of y.
                    emit_y(hsb, tpY, KEVEN)
                    tpN0 = mk("tpN0", 128)
                    nc.tensor.transpose(tpN0[:, 0:128], n_h0[:], IDT)
                    tpN1 = mk("tpN1", 128)
                    nc.tensor.transpose(tpN1[:, 0:128], n_h1[:], IDT)
                    emit_y(hsb, tpY, KODD)

                    hsb = spool.tile([128, 256], BF16, tag="hsb")
                    chain_nhalf(tpN0, zcT, pT, hsb, 0)
                    chain_nhalf(tpN1, zcT, pT, hsb, 1)
                else:
                    emit_y(hsb, tpY, KEVEN)
                    emit_y(hsb, tpY, KODD)

                ys = apool.tile([128, YW], F32, tag="ys")
                nc.scalar.copy(ys[:], tpY[:, 0:YW])
                nc.sync.dma_start(o[t], ys[:])

    nc.compile()
    return nc


def _pack_bat(M):
    """[32, 4*W] -> [128, W]: row 32j+b holds M[b, W*j : W*j+W]."""
    w = M.shape[1] // 4
    return np.ascontiguousarray(
        M.reshape(BC, 4, w).transpose(1, 0, 2).reshape(128, w)
    )


def _prep_shared(w_ih, w_hh, b_ih, b_hh, w_fc, b_fc):
    wihT = w_ih.T.astype(np.float64)  # [768, 3072]
    whhT = w_hh.T.astype(np.float64)  # [1024, 3072]
    wfcT = w_fc.T.astype(np.float64)  # [1024, 768]
    fold = wfcT @ wihT                # [1024, 3072]
    Wr = fold[:, 0:H] + whhT[:, 0:H]
    Wz = fold[:, H : 2 * H] + whhT[:, H : 2 * H]
    Win = fold[:, 2 * H : 3 * H]
    Whn = whhT[:, 2 * H : 3 * H]

    bfold = b_fc.astype(np.float64) @ wihT  # [3072]
    br = bfold[0:H] + b_ih[0:H] + b_hh[0:H]
    bz = bfold[H : 2 * H] + b_ih[H : 2 * H] + b_hh[H : 2 * H]
    bin_ = bfold[2 * H :] + b_ih[2 * H :]
    bhn = b_hh[2 * H :].astype(np.float64)

    blocks = []
    # r|hn interleaved per (k,j) for N=512 pair matmuls
    for k in range(KH):
        for j in range(4):
            blocks.append(Wr[128 * k : 128 * k + 128, 256 * j : 256 * j + 256])
            blocks.append(Whn[128 * k : 128 * k + 128, 256 * j : 256 * j + 256])
    # then z, in blocks (N=256)
    for G in (Wz, Win):
        for k in range(KH):
            for j in range(4):
                blocks.append(G[128 * k : 128 * k + 128, 256 * j : 256 * j + 256])
    WGp = np.concatenate(blocks, axis=1).astype(ml_dtypes.bfloat16)  # [128, 32768]

    yblocks = []
    for k in range(KH):
        for j in range(4):
            yblocks.append(wfcT[128 * k : 128 * k + 128, YW * j : YW * j + YW])
    WFp = np.concatenate(yblocks, axis=1).astype(ml_dtypes.bfloat16)  # [128, 6144]

    ones_col = np.zeros((128, 32), ml_dtypes.bfloat16)
    ones_col[0, :] = 1
    # bias layout: j-paired [br_j | bhn_j] (4x512) then bz (1024), bin (1024)
    bias_row = np.empty(4096, np.float64)
    for j in range(4):
        bias_row[512 * j : 512 * j + 256] = br[256 * j : 256 * j + 256]
        bias_row[512 * j + 256 : 512 * j + 512] = bhn[256 * j : 256 * j + 256]
    bias_row[2048:3072] = bz
    bias_row[3072:4096] = bin_
    bias_col = np.zeros((128, 4096), ml_dtypes.bfloat16)
    bias_col[0, :] = bias_row.astype(ml_dtypes.bfloat16)

    ib32 = np.tile(np.eye(32), (4, 1)).astype(ml_dtypes.bfloat16)  # [128, 32]

    CBp = np.concatenate([WGp, WFp, ones_col, bias_col, ib32], axis=1)  # [128, NB]
    assert CBp.shape[1] == NB
    IDT = np.eye(128, dtype=np.float32)
    return CBp, IDT


def _build_in_maps(inputs):
    src = np.asarray(inputs["src"], np.float32)
    hidden = np.asarray(inputs["hidden"], np.float32)
    w_ih = np.asarray(inputs["w_ih"], np.float32)
    w_hh = np.asarray(inputs["w_hh"], np.float32)
    b_ih = np.asarray(inputs["b_ih"], np.float32)
    b_hh = np.asarray(inputs["b_hh"], np.float32)
    w_fc = np.asarray(inputs["w_fc"], np.float32)
    b_fc = np.asarray(inputs["b_fc"], np.float32)

    CBp, IDT = _prep_shared(w_ih, w_hh, b_ih, b_hh, w_fc, b_fc)

    # step-0 gates on host (f64): from x0=src[0], h0=hidden[0]
    x0 = src[0].astype(np.float64)
    h0 = hidden[0].astype(np.float64)
    gi0 = x0 @ w_ih.T.astype(np.float64) + b_ih.astype(np.float64)
    gh0 = h0 @ w_hh.T.astype(np.float64) + b_hh.astype(np.float64)
    g0r = gi0[:, 0:H] + gh0[:, 0:H]
    g0z = gi0[:, H : 2 * H] + gh0[:, H : 2 * H]
    g0in = gi0[:, 2 * H :]
    g0hn = gh0[:, 2 * H :]

    in_maps = []
    for c in range(NCORES):
        sl = slice(BC * c, BC * (c + 1))
        G0 = np.concatenate(
            [
                _pack_bat(g0r[sl]),
                _pack_bat(g0hn[sl]),
                _pack_bat(g0z[sl]),
                _pack_bat(g0in[sl]),
            ],
            axis=1,
        )  # [128, 1024] in region order r|hn|z|in
        HP0 = _pack_bat(h0[sl])  # [128, 256]
        H0T = np.concatenate(
            [HP0[:, 0:128].T, HP0[:, 128:256].T], axis=1
        )  # transposed-state layout
        CFp = np.concatenate([G0, H0T, IDT, np.ones((128, 256), np.float32)], axis=1).astype(np.float32)
        assert CFp.shape[1] == NF
        in_maps.append(dict(CB=CBp, CF=CFp))
    return in_maps


def kernel(src, tgt, hidden, w_ih, w_hh, b_ih, b_hh, w_fc, b_fc, **_kw):
    global _COMPILED
    b_fc = np.asarray(b_fc, np.float32)

    if _COMPILED is None:
        _COMPILED = _build_nc()
    nc = _COMPILED

    in_maps = _build_in_maps(
        dict(src=src, hidden=hidden, w_ih=w_ih, w_hh=w_hh, b_ih=b_ih,
             b_hh=b_hh, w_fc=w_fc, b_fc=b_fc)
    )

    res = run_bass_kernel_spmd(nc, in_maps, list(range(NCORES)))

    out = np.empty((T, B, O), np.float32)
    for c in range(NCORES):
        sl = slice(BC * c, BC * (c + 1))
        oc = np.asarray(res.results[c]["O"])  # [T, 128, 192]
        out[:, sl, :] = (
            oc.reshape(T, 4, BC, YW).transpose(0, 2, 1, 3).reshape(T, BC, O)
        )
    out += b_fc[None, None, :]
    return out


# revision 13
# speedup vs baseline: 1.1728x; 1.0155x over previous
"""GRU decoder Trainium2 kernel (data-parallel over batch, 8 cores).

Reference (per step t, PyTorch nn.GRU gate order r,z,n):
    gi = x @ w_ih.T + b_ih ; gh = h @ w_hh.T + b_hh
    r = sig(i_r + h_r); z = sig(i_z + h_z); n = tanh(i_n + r * h_n)
    h' = (1-z)*n + z*h ; y = h' @ w_fc.T + b_fc ; x <- y
Shapes: H=1024, O=768, B=256, T=256.  Each core handles 32 batch rows.

Structure (v10 - v9 + warm-clock matmul transposes):
  * x_t = y_{t-1} folds into the hidden-side matmuls, so every recurrent
    matmul contracts over H=1024: regions r, hn (= h_n), z, in (= i_n).
  * The state lives ONLY as hsb = h'^T (bf16, PE lhsT layout).
  * rt = sig(r)*hn (bf16) is ACCUMULATED INTO the gI PSUM by per-quadrant
    diagonal-identity matmuls, so n = tanh(gI) reads PSUM directly - no
    DVE add / extra sem hop on the critical tail.  bf16 rt keeps the
    inject to one cheap round (fp32 would lower to 2 LOW_HIGH passes).
  * The n-side tail is HALVED: tanh / n^T-transpose / vT / hsb' run per
    128-col half into per-bank PSUM tiles, and the next step's r|hn
    matmuls are issued even-chunks-first (even chunks only read the h0
    half of the fresh state), so they start one half earlier.
  * y_t writes its own PSUM bank (no DVE-read-vs-PE-write bank stall on
    the combine) and is split around the n^T transposes to fill PE gaps.
  * Biases seed PSUM via K=1 ones-row matmuls.  One start=True per bank.
  * Step-0 gates come from the host; b_fc is added on the host.
"""

import numpy as np
import ml_dtypes

import concourse.bass as bass
import concourse.bacc as bacc
import concourse.tile as tile
from concourse import mybir
from concourse.bass_utils import run_bass_kernel_spmd

H = 1024
O = 768
B = 256
T = 256
NCORES = 8
BC = B // NCORES  # 32 batch rows per core

KH = H // 128  # 8 contraction chunks
NGATE = 4      # regions r, hn, z, in (issue order)
YW = O // 4    # 192 y cols per quadrant

F32 = mybir.dt.float32
BF16 = mybir.dt.bfloat16
AF = mybir.ActivationFunctionType
ALU = mybir.AluOpType

_COMPILED = None

# bf16 const layout: WG | WF | ONES | BIAS | IB32 | IB128
WG_N = NGATE * KH * 4 * 256   # 32768
WF_N = KH * 4 * YW            # 6144
NB = WG_N + WF_N + 32 + 4096 + 32 + 128  # 43200
# f32 const layout: G0 (r|hn|z|in) | H0T | IDT | ONESF
NF = NGATE * 256 + 256 + 128 + 256  # 1664

KEVEN = (0, 2, 4, 6)
KODD = (1, 3, 5, 7)


def _hslice(hsb, k):
    """lhsT chunk k (h features 128k..128k+128) from packed h'^T tile."""
    c = 128 * (k % 2) + 32 * (k // 2)
    return hsb[:, c : c + 32]


def _build_nc():
    nc = bacc.Bacc("TRN2", target_bir_lowering=False, debug=False, num_devices=NCORES)

    cb = nc.declare_dram_parameter("CB", [128, NB], BF16, isOutput=False)
    cf = nc.declare_dram_parameter("CF", [128, NF], F32, isOutput=False)
    o = nc.declare_dram_parameter("O", [T, 128, YW], F32, isOutput=True)

    with tile.TileContext(nc) as tc:
        with (
            tc.tile_pool(name="wpool", bufs=1) as wpool,
            tc.tile_pool(name="state", bufs=2) as spool,
            tc.tile_pool(name="act", bufs=2) as apool,
            tc.tile_pool(name="gps", bufs=1, space="PSUM") as gpool,
        ):
            CB = wpool.tile([128, NB], BF16, tag="CB")
            CF = wpool.tile([128, NF], F32, tag="CF")
            nc.sync.dma_start(CB[:], cb[:])
            nc.sync.dma_start(CF[:], cf[:])
            WG = CB[:, 0:WG_N]
            WF = CB[:, WG_N : WG_N + WF_N]
            ONES = CB[0:1, WG_N + WF_N : WG_N + WF_N + 32]
            BIAS = CB[0:1, WG_N + WF_N + 32 : WG_N + WF_N + 32 + 4096]
            IB32 = CB[:, WG_N + WF_N + 32 + 4096 : WG_N + WF_N + 32 + 4128]  # 4x I32
            IB128 = CB[:, WG_N + WF_N + 32 + 4128 : NB]  # bf16 I128
            G0 = CF[:, 0 : NGATE * 256]
            H0T = CF[:, NGATE * 256 : NGATE * 256 + 256]
            IDT = CF[:, NGATE * 256 + 256 : NGATE * 256 + 384]
            ONESF = CF[:, NGATE * 256 + 384 : NF]  # all-ones f32 [128,256]

            # PSUM: 8 banks exactly: gA x2 | gZ | gI | tpZ | tpN0 | tpN1 | tpY
            def mk_gA():
                gA = gpool.tile([128, 512], F32, tag="gA", name="gA", bufs=2)
                return gA

            def mk(tag, n):
                # bank-padded (512 f32) so no two PSUM tiles share a bank;
                # hand back a view of the first n cols
                full = gpool.tile([128, 512], F32, tag=tag, name=tag, bufs=1)
                return full[:, 0:n]

            def emit_bias(gA, gZ, gI):
                for j in range(4):
                    nc.tensor.matmul(
                        gA[32 * j : 32 * j + 32, :],
                        ONES[:, 0:32],
                        BIAS[:, 512 * j : 512 * j + 512],
                        start=True, stop=False, tile_position=(0, 32 * j),
                    )
                for gi, gt in ((2, gZ), (3, gI)):
                    for j in range(4):
                        bofs = 1024 * gi + 256 * j
                        nc.tensor.matmul(
                            gt[32 * j : 32 * j + 32, :],
                            ONES[:, 0:32],
                            BIAS[:, bofs : bofs + 256],
                            start=True, stop=False, tile_position=(0, 32 * j),
                        )

            def emit_A(hsb, gA):
                # r|hn pair as single N=512 matmuls; even chunks first so
                # the round 0 only waits on the h0 half of the new state.
                for i, k in enumerate(KEVEN + KODD):
                    lhsT = _hslice(hsb, k)
                    for j in range(4):
                        wofs = (k * 4 + j) * 512
                        nc.tensor.matmul(
                            gA[32 * j : 32 * j + 32, :],
                            lhsT,
                            WG[:, wofs : wofs + 512],
                            start=False,
                            stop=(i == KH - 1),
                            tile_position=(0, 32 * j),
                        )

            def emit_ZI(hsb, gt, gi, stop_last):
                for k in range(KH):
                    lhsT = _hslice(hsb, k)
                    for j in range(4):
                        wofs = 16384 + ((gi * KH + k) * 4 + j) * 256
                        nc.tensor.matmul(
                            gt[32 * j : 32 * j + 32, :],
                            lhsT,
                            WG[:, wofs : wofs + 256],
                            start=False,
                            stop=(stop_last and k == KH - 1),
                            tile_position=(0, 32 * j),
                        )

            def emit_inject(gI, rtb):
                # gI += I32^T @ rt per quadrant (bf16, diagonal PE tiles)
                for j in range(4):
                    nc.tensor.matmul(
                        gI[32 * j : 32 * j + 32, :],
                        IB32[32 * j : 32 * j + 32, 0:32],
                        rtb[32 * j : 32 * j + 32, :],
                        start=False,
                        stop=True,
                        tile_position=(32 * j, 32 * j),
                    )

            def emit_y(hsb_t, tpY, kset):
                for k in kset:
                    lhsT = _hslice(hsb_t, k)
                    for j in range(4):
                        wofs = (k * 4 + j) * YW
                        nc.tensor.matmul(
                            tpY[32 * j : 32 * j + 32, 0:YW],
                            lhsT,
                            WF[:, wofs : wofs + YW],
                            start=(k == 0),
                            stop=(k == KH - 1),
                            tile_position=(0, 32 * j),
                        )

            def chain_zside(zs, tpZ, hsb_prev):
                """zs^T (PE) was emitted by caller; zc^T and p^T on DVE
                (early, off the critical tail)."""
                zcT = apool.tile([128, 256], F32, tag="zcT")
                nc.vector.tensor_tensor(zcT[:], ONESF, tpZ[:], ALU.subtract)
                pT = apool.tile([128, 256], F32, tag="pT")
                nc.vector.tensor_tensor(pT[:], tpZ[:], hsb_prev, ALU.mult)
                return zcT, pT

            def chain_nhalf(tpN, zcT, pT, hsb2, hh):
                """vT = n^T * zc^T ; hsb'[half] = vT + pT[half]  (DVE)."""
                sl = slice(128 * hh, 128 * hh + 128)
                vT = apool.tile([128, 128], F32, tag=f"vT{hh}")
                nc.vector.tensor_tensor(vT[:], tpN[:, 0:128], zcT[:, sl], ALU.mult)
                nc.vector.tensor_tensor(hsb2[:, sl], vT[:], pT[:, sl], ALU.add)

            # ---- step 0: gates computed host-side (biases included) ----
            rs0 = apool.tile([128, 256], F32, tag="rs")
            nc.scalar.activation(rs0[:], G0[:, 0:256], AF.Sigmoid)
            zs0 = apool.tile([128, 256], BF16, tag="zs")
            nc.scalar.activation(zs0[:], G0[:, 512:768], AF.Sigmoid)
            rt0 = apool.tile([128, 256], F32, tag="rt0")
            nc.vector.tensor_tensor(rt0[:], rs0[:], G0[:, 256:512], ALU.mult)
            ns0 = apool.tile([128, 256], F32, tag="ns0")
            nc.vector.tensor_tensor(ns0[:], rt0[:], G0[:, 768:1024], ALU.add)
            n0 = apool.tile([128, 256], BF16, tag="n0")
            nc.scalar.activation(n0[:], ns0[:], AF.Tanh)
            tpZ = mk("tpZ", 256)
            nc.tensor.matmul(tpZ[:, 0:128], zs0[:, 0:128], IB128, start=True, stop=True)
            nc.tensor.matmul(tpZ[:, 128:256], zs0[:, 128:256], IB128, start=True, stop=True)
            tpN0 = mk("tpN0", 128)
            tpN1 = mk("tpN1", 128)
            nc.tensor.matmul(tpN0[:, 0:128], n0[:, 0:128], IB128, start=True, stop=True)
            nc.tensor.matmul(tpN1[:, 0:128], n0[:, 128:256], IB128, start=True, stop=True)
            zcT, pT = chain_zside(zs0, tpZ, H0T)
            hsb = spool.tile([128, 256], BF16, tag="hsb")
            chain_nhalf(tpN0, zcT, pT, hsb, 0)
            chain_nhalf(tpN1, zcT, pT, hsb, 1)

            for t in range(T):
                last = t == T - 1
                tpY = mk("tpY", YW)
                if not last:
                    gA = mk_gA()         # r | hn
                    gZ = mk("gZ", 256)
                    gI = mk("gI", 256)
                    emit_bias(gA, gZ, gI)
                    emit_A(hsb, gA)
                    emit_ZI(hsb, gZ, 0, stop_last=True)

                    rs = apool.tile([128, 256], F32, tag="rs")
                    nc.scalar.activation(rs[:], gA[:, 0:256], AF.Sigmoid)
                    zs = apool.tile([128, 256], BF16, tag="zs")
                    nc.scalar.activation(zs[:], gZ[:], AF.Sigmoid)
                    rtb = apool.tile([128, 256], BF16, tag="rtb")
                    nc.vector.tensor_tensor(rtb[:], rs[:], gA[:, 256:512], ALU.mult)

                    emit_ZI(hsb, gI, 1, stop_last=False)
                    emit_inject(gI, rtb)

                    tpZ = mk("tpZ", 256)
                    nc.tensor.matmul(tpZ[:, 0:128], zs[:, 0:128], IB128, start=True, stop=True)
                    nc.tensor.matmul(tpZ[:, 128:256], zs[:, 128:256], IB128, start=True, stop=True)

                    n_h0 = apool.tile([128, 128], BF16, tag="n_h0")
                    nc.scalar.activation(n_h0[:], gI[:, 0:128], AF.Tanh)
                    n_h1 = apool.tile([128, 128], BF16, tag="n_h1")
                    nc.scalar.activation(n_h1[:], gI[:, 128:256], AF.Tanh)

                    zcT, pT = chain_zside(zs, tpZ, hsb[:])

                    # PE: fill with half of y while tanh runs, then the
                    # n^T transposes, then the rest of y.
                    emit_y(hsb, tpY, KEVEN)
                    tpN0 = mk("tpN0", 128)
                    nc.tensor.matmul(tpN0[:, 0:128], n_h0[:], IB128, start=True, stop=True)
                    tpN1 = mk("tpN1", 128)
                    nc.tensor.matmul(tpN1[:, 0:128], n_h1[:], IB128, start=True, stop=True)
                    emit_y(hsb, tpY, KODD)

                    hsb = spool.tile([128, 256], BF16, tag="hsb")
                    chain_nhalf(tpN0, zcT, pT, hsb, 0)
                    chain_nhalf(tpN1, zcT, pT, hsb, 1)
                else:
                    emit_y(hsb, tpY, KEVEN)
                    emit_y(hsb, tpY, KODD)

                ys = apool.tile([128, YW], F32, tag="ys")
                nc.scalar.copy(ys[:], tpY[:, 0:YW])
                nc.sync.dma_start(o[t], ys[:])

    nc.compile()
    return nc


def _pack_bat(M):
    """[32, 4*W] -> [128, W]: row 32j+b holds M[b, W*j : W*j+W]."""
    w = M.shape[1] // 4
    return np.ascontiguousarray(
        M.reshape(BC, 4, w).transpose(1, 0, 2).reshape(128, w)
    )


def _prep_shared(w_ih, w_hh, b_ih, b_hh, w_fc, b_fc):
    wihT = w_ih.T.astype(np.float64)  # [768, 3072]
    whhT = w_hh.T.astype(np.float64)  # [1024, 3072]
    wfcT = w_fc.T.astype(np.float64)  # [1024, 768]
    fold = wfcT @ wihT                # [1024, 3072]
    Wr = fold[:, 0:H] + whhT[:, 0:H]
    Wz = fold[:, H : 2 * H] + whhT[:, H : 2 * H]
    Win = fold[:, 2 * H : 3 * H]
    Whn = whhT[:, 2 * H : 3 * H]

    bfold = b_fc.astype(np.float64) @ wihT  # [3072]
    br = bfold[0:H] + b_ih[0:H] + b_hh[0:H]
    bz = bfold[H : 2 * H] + b_ih[H : 2 * H] + b_hh[H : 2 * H]
    bin_ = bfold[2 * H :] + b_ih[2 * H :]
    bhn = b_hh[2 * H :].astype(np.float64)

    blocks = []
    # r|hn interleaved per (k,j) for N=512 pair matmuls
    for k in range(KH):
        for j in range(4):
            blocks.append(Wr[128 * k : 128 * k + 128, 256 * j : 256 * j + 256])
            blocks.append(Whn[128 * k : 128 * k + 128, 256 * j : 256 * j + 256])
    # then z, in blocks (N=256)
    for G in (Wz, Win):
        for k in range(KH):
            for j in range(4):
                blocks.append(G[128 * k : 128 * k + 128, 256 * j : 256 * j + 256])
    WGp = np.concatenate(blocks, axis=1).astype(ml_dtypes.bfloat16)  # [128, 32768]

    yblocks = []
    for k in range(KH):
        for j in range(4):
            yblocks.append(wfcT[128 * k : 128 * k + 128, YW * j : YW * j + YW])
    WFp = np.concatenate(yblocks, axis=1).astype(ml_dtypes.bfloat16)  # [128, 6144]

    ones_col = np.zeros((128, 32), ml_dtypes.bfloat16)
    ones_col[0, :] = 1
    # bias layout: j-paired [br_j | bhn_j] (4x512) then bz (1024), bin (1024)
    bias_row = np.empty(4096, np.float64)
    for j in range(4):
        bias_row[512 * j : 512 * j + 256] = br[256 * j : 256 * j + 256]
        bias_row[512 * j + 256 : 512 * j + 512] = bhn[256 * j : 256 * j + 256]
    bias_row[2048:3072] = bz
    bias_row[3072:4096] = bin_
    bias_col = np.zeros((128, 4096), ml_dtypes.bfloat16)
    bias_col[0, :] = bias_row.astype(ml_dtypes.bfloat16)

    ib32 = np.tile(np.eye(32), (4, 1)).astype(ml_dtypes.bfloat16)  # [128, 32]
    ib128 = np.eye(128).astype(ml_dtypes.bfloat16)  # [128, 128]

    CBp = np.concatenate([WGp, WFp, ones_col, bias_col, ib32, ib128], axis=1)  # [128, NB]
    assert CBp.shape[1] == NB
    IDT = np.eye(128, dtype=np.float32)
    return CBp, IDT


def _build_in_maps(inputs):
    src = np.asarray(inputs["src"], np.float32)
    hidden = np.asarray(inputs["hidden"], np.float32)
    w_ih = np.asarray(inputs["w_ih"], np.float32)
    w_hh = np.asarray(inputs["w_hh"], np.float32)
    b_ih = np.asarray(inputs["b_ih"], np.float32)
    b_hh = np.asarray(inputs["b_hh"], np.float32)
    w_fc = np.asarray(inputs["w_fc"], np.float32)
    b_fc = np.asarray(inputs["b_fc"], np.float32)

    CBp, IDT = _prep_shared(w_ih, w_hh, b_ih, b_hh, w_fc, b_fc)

    # step-0 gates on host (f64): from x0=src[0], h0=hidden[0]
    x0 = src[0].astype(np.float64)
    h0 = hidden[0].astype(np.float64)
    gi0 = x0 @ w_ih.T.astype(np.float64) + b_ih.astype(np.float64)
    gh0 = h0 @ w_hh.T.astype(np.float64) + b_hh.astype(np.float64)
    g0r = gi0[:, 0:H] + gh0[:, 0:H]
    g0z = gi0[:, H : 2 * H] + gh0[:, H : 2 * H]
    g0in = gi0[:, 2 * H :]
    g0hn = gh0[:, 2 * H :]

    in_maps = []
    for c in range(NCORES):
        sl = slice(BC * c, BC * (c + 1))
        G0 = np.concatenate(
            [
                _pack_bat(g0r[sl]),
                _pack_bat(g0hn[sl]),
                _pack_bat(g0z[sl]),
                _pack_bat(g0in[sl]),
            ],
            axis=1,
        )  # [128, 1024] in region order r|hn|z|in
        HP0 = _pack_bat(h0[sl])  # [128, 256]
        H0T = np.concatenate(
            [HP0[:, 0:128].T, HP0[:, 128:256].T], axis=1
        )  # transposed-state layout
        CFp = np.concatenate([G0, H0T, IDT, np.ones((128, 256), np.float32)], axis=1).astype(np.float32)
        assert CFp.shape[1] == NF
        in_maps.append(dict(CB=CBp, CF=CFp))
    return in_maps


def kernel(src, tgt, hidden, w_ih, w_hh, b_ih, b_hh, w_fc, b_fc, **_kw):
    global _COMPILED
    b_fc = np.asarray(b_fc, np.float32)

    if _COMPILED is None:
        _COMPILED = _build_nc()
    nc = _COMPILED

    in_maps = _build_in_maps(
        dict(src=src, hidden=hidden, w_ih=w_ih, w_hh=w_hh, b_ih=b_ih,
             b_hh=b_hh, w_fc=w_fc, b_fc=b_fc)
    )

    res = run_bass_kernel_spmd(nc, in_maps, list(range(NCORES)))

    out = np.empty((T, B, O), np.float32)
    for c in range(NCORES):
        sl = slice(BC * c, BC * (c + 1))
        oc = np.asarray(res.results[c]["O"])  # [T, 128, 192]
        out[:, sl, :] = (
            oc.reshape(T, 4, BC, YW).transpose(0, 2, 1, 3).reshape(T, BC, O)
        )
    out += b_fc[None, None, :]
    return out


# revision 14
# speedup vs baseline: 1.2199x; 1.0402x over previous
"""GRU decoder Trainium2 kernel (data-parallel over batch, 8 cores).

Reference (per step t, PyTorch nn.GRU gate order r,z,n):
    gi = x @ w_ih.T + b_ih ; gh = h @ w_hh.T + b_hh
    r = sig(i_r + h_r); z = sig(i_z + h_z); n = tanh(i_n + r * h_n)
    h' = (1-z)*n + z*h ; y = h' @ w_fc.T + b_fc ; x <- y
Shapes: H=1024, O=768, B=256, T=256.  Each core handles 32 batch rows.

Structure (v11 - v10 + bias-in-stall-windows + per-half pT):
  * x_t = y_{t-1} folds into the hidden-side matmuls, so every recurrent
    matmul contracts over H=1024: regions r, hn (= h_n), z, in (= i_n).
  * The state lives ONLY as hsb = h'^T (bf16, PE lhsT layout).
  * rt = sig(r)*hn (bf16) is ACCUMULATED INTO the gI PSUM by per-quadrant
    diagonal-identity matmuls, so n = tanh(gI) reads PSUM directly - no
    DVE add / extra sem hop on the critical tail.  bf16 rt keeps the
    inject to one cheap round (fp32 would lower to 2 LOW_HIGH passes).
  * The n-side tail is HALVED: tanh / n^T-transpose / vT / hsb' run per
    128-col half into per-bank PSUM tiles, and the next step's r|hn
    matmuls are issued even-chunks-first (even chunks only read the h0
    half of the fresh state), so they start one half earlier.
  * y_t writes its own PSUM bank (no DVE-read-vs-PE-write bank stall on
    the combine) and is split around the n^T transposes to fill PE gaps.
  * Biases seed PSUM via K=1 ones-row matmuls.  One start=True per bank.
  * Step-0 gates come from the host; b_fc is added on the host.
"""

import numpy as np
import ml_dtypes

import concourse.bass as bass
import concourse.bacc as bacc
import concourse.tile as tile
from concourse import mybir
from concourse.bass_utils import run_bass_kernel_spmd

H = 1024
O = 768
B = 256
T = 256
NCORES = 8
BC = B // NCORES  # 32 batch rows per core

KH = H // 128  # 8 contraction chunks
NGATE = 4      # regions r, hn, z, in (issue order)
YW = O // 4    # 192 y cols per quadrant

F32 = mybir.dt.float32
BF16 = mybir.dt.bfloat16
AF = mybir.ActivationFunctionType
ALU = mybir.AluOpType

_COMPILED = None

# bf16 const layout: WG | WF | ONES | BIAS | IB32 | IB128
WG_N = NGATE * KH * 4 * 256   # 32768
WF_N = KH * 4 * YW            # 6144
NB = WG_N + WF_N + 32 + 4096 + 32 + 128  # 43200
# f32 const layout: G0 (r|hn|z|in) | H0T | IDT | ONESF
NF = NGATE * 256 + 256 + 128 + 256  # 1664

KEVEN = (0, 2, 4, 6)
KODD = (1, 3, 5, 7)


def _hslice(hsb, k):
    """lhsT chunk k (h features 128k..128k+128) from packed h'^T tile."""
    c = 128 * (k % 2) + 32 * (k // 2)
    return hsb[:, c : c + 32]


def _build_nc():
    nc = bacc.Bacc("TRN2", target_bir_lowering=False, debug=False, num_devices=NCORES)

    cb = nc.declare_dram_parameter("CB", [128, NB], BF16, isOutput=False)
    cf = nc.declare_dram_parameter("CF", [128, NF], F32, isOutput=False)
    o = nc.declare_dram_parameter("O", [T, 128, YW], F32, isOutput=True)

    with tile.TileContext(nc) as tc:
        with (
            tc.tile_pool(name="wpool", bufs=1) as wpool,
            tc.tile_pool(name="state", bufs=2) as spool,
            tc.tile_pool(name="act", bufs=2) as apool,
            tc.tile_pool(name="gps", bufs=1, space="PSUM") as gpool,
        ):
            CB = wpool.tile([128, NB], BF16, tag="CB")
            CF = wpool.tile([128, NF], F32, tag="CF")
            nc.sync.dma_start(CB[:], cb[:])
            nc.sync.dma_start(CF[:], cf[:])
            WG = CB[:, 0:WG_N]
            WF = CB[:, WG_N : WG_N + WF_N]
            ONES = CB[0:1, WG_N + WF_N : WG_N + WF_N + 32]
            BIAS = CB[0:1, WG_N + WF_N + 32 : WG_N + WF_N + 32 + 4096]
            IB32 = CB[:, WG_N + WF_N + 32 + 4096 : WG_N + WF_N + 32 + 4128]  # 4x I32
            IB128 = CB[:, WG_N + WF_N + 32 + 4128 : NB]  # bf16 I128
            G0 = CF[:, 0 : NGATE * 256]
            H0T = CF[:, NGATE * 256 : NGATE * 256 + 256]
            IDT = CF[:, NGATE * 256 + 256 : NGATE * 256 + 384]
            ONESF = CF[:, NGATE * 256 + 384 : NF]  # all-ones f32 [128,256]

            # PSUM: 8 banks exactly: gA x2 | gZ | gI | tpZ | tpN0 | tpN1 | tpY
            def mk_gA():
                gA = gpool.tile([128, 512], F32, tag="gA", name="gA", bufs=2)
                return gA

            def mk(tag, n):
                # bank-padded (512 f32) so no two PSUM tiles share a bank;
                # hand back a view of the first n cols
                full = gpool.tile([128, 512], F32, tag=tag, name=tag, bufs=1)
                return full[:, 0:n]

            def emit_biasA(gA):
                for j in range(4):
                    nc.tensor.matmul(
                        gA[32 * j : 32 * j + 32, :],
                        ONES[:, 0:32],
                        BIAS[:, 512 * j : 512 * j + 512],
                        start=True, stop=False, tile_position=(0, 32 * j),
                    )

            def emit_biasZI(gt, gi):
                for j in range(4):
                    bofs = 1024 * gi + 256 * j
                    nc.tensor.matmul(
                        gt[32 * j : 32 * j + 32, :],
                        ONES[:, 0:32],
                        BIAS[:, bofs : bofs + 256],
                        start=True, stop=False, tile_position=(0, 32 * j),
                    )

            def mk_gates():
                return mk_gA(), mk("gZ", 256), mk("gI", 256)

            def emit_A(hsb, gA):
                # r|hn pair as single N=512 matmuls; even chunks first so
                # the round 0 only waits on the h0 half of the new state.
                for i, k in enumerate(KEVEN + KODD):
                    lhsT = _hslice(hsb, k)
                    for j in range(4):
                        wofs = (k * 4 + j) * 512
                        nc.tensor.matmul(
                            gA[32 * j : 32 * j + 32, :],
                            lhsT,
                            WG[:, wofs : wofs + 512],
                            start=False,
                            stop=(i == KH - 1),
                            tile_position=(0, 32 * j),
                        )

            def emit_ZI(hsb, gt, gi, stop_last):
                for k in range(KH):
                    lhsT = _hslice(hsb, k)
                    for j in range(4):
                        wofs = 16384 + ((gi * KH + k) * 4 + j) * 256
                        nc.tensor.matmul(
                            gt[32 * j : 32 * j + 32, :],
                            lhsT,
                            WG[:, wofs : wofs + 256],
                            start=False,
                            stop=(stop_last and k == KH - 1),
                            tile_position=(0, 32 * j),
                        )

            def emit_inject(gI, rtb):
                # gI += I32^T @ rt per quadrant (bf16, diagonal PE tiles)
                for j in range(4):
                    nc.tensor.matmul(
                        gI[32 * j : 32 * j + 32, :],
                        IB32[32 * j : 32 * j + 32, 0:32],
                        rtb[32 * j : 32 * j + 32, :],
                        start=False,
                        stop=True,
                        tile_position=(32 * j, 32 * j),
                    )

            def emit_y(hsb_t, tpY, kset):
                for k in kset:
                    lhsT = _hslice(hsb_t, k)
                    for j in range(4):
                        wofs = (k * 4 + j) * YW
                        nc.tensor.matmul(
                            tpY[32 * j : 32 * j + 32, 0:YW],
                            lhsT,
                            WF[:, wofs : wofs + YW],
                            start=(k == 0),
                            stop=(k == KH - 1),
                            tile_position=(0, 32 * j),
                        )

            def chain_zside(zs, tpZ, hsb_prev):
                """zc^T on DVE (early, off the critical tail)."""
                zcT = apool.tile([128, 256], F32, tag="zcT")
                nc.vector.tensor_tensor(zcT[:], ONESF, tpZ[:], ALU.subtract)
                return zcT, (tpZ, hsb_prev)

            def chain_nhalf(tpN, zcT, pz, hsb2, hh):
                """p^T = z^T*h^T ; vT = n^T * zc^T ; hsb'[half] = vT + pT
                (DVE, per half so h0 completes first)."""
                tpZ, hsb_prev = pz
                sl = slice(128 * hh, 128 * hh + 128)
                pT = apool.tile([128, 128], F32, tag=f"pT{hh}")
                nc.vector.tensor_tensor(pT[:], tpZ[:, sl], hsb_prev[:, sl], ALU.mult)
                vT = apool.tile([128, 128], F32, tag=f"vT{hh}")
                nc.vector.tensor_tensor(vT[:], tpN[:, 0:128], zcT[:, sl], ALU.mult)
                nc.vector.tensor_tensor(hsb2[:, sl], vT[:], pT[:], ALU.add)

            # ---- step 0: gates computed host-side (biases included) ----
            rs0 = apool.tile([128, 256], F32, tag="rs")
            nc.scalar.activation(rs0[:], G0[:, 0:256], AF.Sigmoid)
            zs0 = apool.tile([128, 256], BF16, tag="zs")
            nc.scalar.activation(zs0[:], G0[:, 512:768], AF.Sigmoid)
            rt0 = apool.tile([128, 256], F32, tag="rt0")
            nc.vector.tensor_tensor(rt0[:], rs0[:], G0[:, 256:512], ALU.mult)
            ns0 = apool.tile([128, 256], F32, tag="ns0")
            nc.vector.tensor_tensor(ns0[:], rt0[:], G0[:, 768:1024], ALU.add)
            n0 = apool.tile([128, 256], BF16, tag="n0")
            nc.scalar.activation(n0[:], ns0[:], AF.Tanh)
            tpZ = mk("tpZ", 256)
            nc.tensor.matmul(tpZ[:, 0:128], zs0[:, 0:128], IB128, start=True, stop=True)
            nc.tensor.matmul(tpZ[:, 128:256], zs0[:, 128:256], IB128, start=True, stop=True)
            tpN0 = mk("tpN0", 128)
            tpN1 = mk("tpN1", 128)
            nc.tensor.matmul(tpN0[:, 0:128], n0[:, 0:128], IB128, start=True, stop=True)
            nc.tensor.matmul(tpN1[:, 0:128], n0[:, 128:256], IB128, start=True, stop=True)
            zcT, pz = chain_zside(zs0, tpZ, H0T)
            hsb = spool.tile([128, 256], BF16, tag="hsb")
            chain_nhalf(tpN0, zcT, pz, hsb, 0)
            chain_nhalf(tpN1, zcT, pz, hsb, 1)

            # gates for step 1: bias seeded up front
            pend = mk_gates()
            emit_biasA(pend[0])
            emit_biasZI(pend[1], 2)
            emit_biasZI(pend[2], 3)

            for t in range(T):
                last = t == T - 1
                tpY = mk("tpY", YW)
                if not last:
                    gA, gZ, gI = pend
                    emit_A(hsb, gA)
                    emit_ZI(hsb, gZ, 0, stop_last=True)

                    rs = apool.tile([128, 256], F32, tag="rs")
                    nc.scalar.activation(rs[:], gA[:, 0:256], AF.Sigmoid)
                    zs = apool.tile([128, 256], BF16, tag="zs")
                    nc.scalar.activation(zs[:], gZ[:], AF.Sigmoid)
                    rtb = apool.tile([128, 256], BF16, tag="rtb")
                    nc.vector.tensor_tensor(rtb[:], rs[:], gA[:, 256:512], ALU.mult)

                    emit_ZI(hsb, gI, 1, stop_last=False)
                    emit_inject(gI, rtb)

                    tpZ = mk("tpZ", 256)
                    nc.tensor.matmul(tpZ[:, 0:128], zs[:, 0:128], IB128, start=True, stop=True)
                    nc.tensor.matmul(tpZ[:, 128:256], zs[:, 128:256], IB128, start=True, stop=True)

                    n_h0 = apool.tile([128, 128], BF16, tag="n_h0")
                    nc.scalar.activation(n_h0[:], gI[:, 0:128], AF.Tanh)
                    n_h1 = apool.tile([128, 128], BF16, tag="n_h1")
                    nc.scalar.activation(n_h1[:], gI[:, 128:256], AF.Tanh)

                    zcT, pz = chain_zside(zs, tpZ, hsb[:])

                    # PE: fill with half of y while tanh runs; the next
                    # step's bias seeds fill the tanh->transpose stalls.
                    emit_y(hsb, tpY, KEVEN)
                    if t < T - 2:
                        pend = mk_gates()
                        emit_biasA(pend[0])
                    tpN0 = mk("tpN0", 128)
                    nc.tensor.matmul(tpN0[:, 0:128], n_h0[:], IB128, start=True, stop=True)
                    if t < T - 2:
                        emit_biasZI(pend[1], 2)
                    tpN1 = mk("tpN1", 128)
                    nc.tensor.matmul(tpN1[:, 0:128], n_h1[:], IB128, start=True, stop=True)
                    if t < T - 2:
                        emit_biasZI(pend[2], 3)
                    emit_y(hsb, tpY, KODD)

                    hsb = spool.tile([128, 256], BF16, tag="hsb")
                    chain_nhalf(tpN0, zcT, pz, hsb, 0)
                    chain_nhalf(tpN1, zcT, pz, hsb, 1)
                else:
                    emit_y(hsb, tpY, KEVEN)
                    emit_y(hsb, tpY, KODD)

                ys = apool.tile([128, YW], F32, tag="ys")
                nc.scalar.copy(ys[:], tpY[:, 0:YW])
                nc.sync.dma_start(o[t], ys[:])

    nc.compile()
    return nc


def _pack_bat(M):
    """[32, 4*W] -> [128, W]: row 32j+b holds M[b, W*j : W*j+W]."""
    w = M.shape[1] // 4
    return np.ascontiguousarray(
        M.reshape(BC, 4, w).transpose(1, 0, 2).reshape(128, w)
    )


def _prep_shared(w_ih, w_hh, b_ih, b_hh, w_fc, b_fc):
    wihT = w_ih.T.astype(np.float64)  # [768, 3072]
    whhT = w_hh.T.astype(np.float64)  # [1024, 3072]
    wfcT = w_fc.T.astype(np.float64)  # [1024, 768]
    fold = wfcT @ wihT                # [1024, 3072]
    Wr = fold[:, 0:H] + whhT[:, 0:H]
    Wz = fold[:, H : 2 * H] + whhT[:, H : 2 * H]
    Win = fold[:, 2 * H : 3 * H]
    Whn = whhT[:, 2 * H : 3 * H]

    bfold = b_fc.astype(np.float64) @ wihT  # [3072]
    br = bfold[0:H] + b_ih[0:H] + b_hh[0:H]
    bz = bfold[H : 2 * H] + b_ih[H : 2 * H] + b_hh[H : 2 * H]
    bin_ = bfold[2 * H :] + b_ih[2 * H :]
    bhn = b_hh[2 * H :].astype(np.float64)

    blocks = []
    # r|hn interleaved per (k,j) for N=512 pair matmuls
    for k in range(KH):
        for j in range(4):
            blocks.append(Wr[128 * k : 128 * k + 128, 256 * j : 256 * j + 256])
            blocks.append(Whn[128 * k : 128 * k + 128, 256 * j : 256 * j + 256])
    # then z, in blocks (N=256)
    for G in (Wz, Win):
        for k in range(KH):
            for j in range(4):
                blocks.append(G[128 * k : 128 * k + 128, 256 * j : 256 * j + 256])
    WGp = np.concatenate(blocks, axis=1).astype(ml_dtypes.bfloat16)  # [128, 32768]

    yblocks = []
    for k in range(KH):
        for j in range(4):
            yblocks.append(wfcT[128 * k : 128 * k + 128, YW * j : YW * j + YW])
    WFp = np.concatenate(yblocks, axis=1).astype(ml_dtypes.bfloat16)  # [128, 6144]

    ones_col = np.zeros((128, 32), ml_dtypes.bfloat16)
    ones_col[0, :] = 1
    # bias layout: j-paired [br_j | bhn_j] (4x512) then bz (1024), bin (1024)
    bias_row = np.empty(4096, np.float64)
    for j in range(4):
        bias_row[512 * j : 512 * j + 256] = br[256 * j : 256 * j + 256]
        bias_row[512 * j + 256 : 512 * j + 512] = bhn[256 * j : 256 * j + 256]
    bias_row[2048:3072] = bz
    bias_row[3072:4096] = bin_
    bias_col = np.zeros((128, 4096), ml_dtypes.bfloat16)
    bias_col[0, :] = bias_row.astype(ml_dtypes.bfloat16)

    ib32 = np.tile(np.eye(32), (4, 1)).astype(ml_dtypes.bfloat16)  # [128, 32]
    ib128 = np.eye(128).astype(ml_dtypes.bfloat16)  # [128, 128]

    CBp = np.concatenate([WGp, WFp, ones_col, bias_col, ib32, ib128], axis=1)  # [128, NB]
    assert CBp.shape[1] == NB
    IDT = np.eye(128, dtype=np.float32)
    return CBp, IDT


def _build_in_maps(inputs):
    src = np.asarray(inputs["src"], np.float32)
    hidden = np.asarray(inputs["hidden"], np.float32)
    w_ih = np.asarray(inputs["w_ih"], np.float32)
    w_hh = np.asarray(inputs["w_hh"], np.float32)
    b_ih = np.asarray(inputs["b_ih"], np.float32)
    b_hh = np.asarray(inputs["b_hh"], np.float32)
    w_fc = np.asarray(inputs["w_fc"], np.float32)
    b_fc = np.asarray(inputs["b_fc"], np.float32)

    CBp, IDT = _prep_shared(w_ih, w_hh, b_ih, b_hh, w_fc, b_fc)

    # step-0 gates on host (f64): from x0=src[0], h0=hidden[0]
    x0 = src[0].astype(np.float64)
    h0 = hidden[0].astype(np.float64)
    gi0 = x0 @ w_ih.T.astype(np.float64) + b_ih.astype(np.float64)
    gh0 = h0 @ w_hh.T.astype(np.float64) + b_hh.astype(np.float64)
    g0r = gi0[:, 0:H] + gh0[:, 0:H]
    g0z = gi0[:, H : 2 * H] + gh0[:, H : 2 * H]
    g0in = gi0[:, 2 * H :]
    g0hn = gh0[:, 2 * H :]

    in_maps = []
    for c in range(NCORES):
        sl = slice(BC * c, BC * (c + 1))
        G0 = np.concatenate(
            [
                _pack_bat(g0r[sl]),
                _pack_bat(g0hn[sl]),
                _pack_bat(g0z[sl]),
                _pack_bat(g0in[sl]),
            ],
            axis=1,
        )  # [128, 1024] in region order r|hn|z|in
        HP0 = _pack_bat(h0[sl])  # [128, 256]
        H0T = np.concatenate(
            [HP0[:, 0:128].T, HP0[:, 128:256].T], axis=1
        )  # transposed-state layout
        CFp = np.concatenate([G0, H0T, IDT, np.ones((128, 256), np.float32)], axis=1).astype(np.float32)
        assert CFp.shape[1] == NF
        in_maps.append(dict(CB=CBp, CF=CFp))
    return in_maps


def kernel(src, tgt, hidden, w_ih, w_hh, b_ih, b_hh, w_fc, b_fc, **_kw):
    global _COMPILED
    b_fc = np.asarray(b_fc, np.float32)

    if _COMPILED is None:
        _COMPILED = _build_nc()
    nc = _COMPILED

    in_maps = _build_in_maps(
        dict(src=src, hidden=hidden, w_ih=w_ih, w_hh=w_hh, b_ih=b_ih,
             b_hh=b_hh, w_fc=w_fc, b_fc=b_fc)
    )

    res = run_bass_kernel_spmd(nc, in_maps, list(range(NCORES)))

    out = np.empty((T, B, O), np.float32)
    for c in range(NCORES):
        sl = slice(BC * c, BC * (c + 1))
        oc = np.asarray(res.results[c]["O"])  # [T, 128, 192]
        out[:, sl, :] = (
            oc.reshape(T, 4, BC, YW).transpose(0, 2, 1, 3).reshape(T, BC, O)
        )
    out += b_fc[None, None, :]
    return out
